# revision 67
# baseline (speedup 1.0000x reference)
"""Trainium2 Bass kernel for nn_EnhancedStateEncoder (6-layer dense transformer).

Strategy: data-parallel over batch across 8 NeuronCores (2 batches/core).
 - Embedding + sinusoidal pos-emb folded on host (cheap gather).
 - Alibi bias is rank-1 across heads (slopes are exact powers of two:
   bias[h] = 2^-h * bias[0]), so only bias[0]^T is stored (bf16, prescaled
   by 1/sqrt(d)); it is added to Q@K^T PSUM via a per-head scaled-identity
   matmul on the TensorEngine.
 - QK^T and the first MLP matmul stream as float32r (tf32-like, 1 cycle/row
   for moving dim >= 256 vs 4 for fp32).
 - Attention computed in S^T layout [j(part), i(free)] in [128,512] PSUM
   units, software-pipelined two units deep so the PE never waits for the
   ScalarE exp; a ones-column appended to V yields softmax denominators.
 - Per-head output epilogue (PSUM drain, transpose, 1/denom scale) is
   deferred into the next head's unit loop to avoid PE stalls.
 - LayerNorm via bn_stats/bn_aggr; rsqrt as exp(-0.5*ln(var+eps)).
 - LN2's affine is folded into the MLP's first matmul on the host.
 - Phases alternate between the two batches per core so LN (vector/scalar)
   work of one batch hides under attention/MLP (PE) work of the other.
 - All layer weights are DMA'd into SBUF once, up front.
"""

import math
import os
from contextlib import ExitStack

import numpy as np
import ml_dtypes

import concourse.bass as bass
import concourse.mybir as mybir
import concourse.tile as tile
from concourse.bass_utils import run_bass_kernel_spmd
from concourse.masks import make_identity

F32 = mybir.dt.float32
F32R = mybir.dt.float32r
BF16 = mybir.dt.bfloat16

B, S, D, H, HD, L, H2 = 16, 1024, 256, 8, 32, 6, 1024
NC = 8            # cores
BL = B // NC      # batches per core = 2
T = BL * S        # tokens per core = 2048
NCH = T // 128    # 128-token chunks per core = 16
SCALE = 1.0 / math.sqrt(HD)
LN_EPS = 1e-5
GRID = 32

_cache = {}


def _alibi_bias0T():
    """biasT0[p, jc, i] = bias[h=0, i, jc*128+p] / SCALE, bf16.

    bias[h] == 2^-h * bias[0] exactly (slopes are powers of two), so only
    head 0 is stored; per-head scaling happens in the identity matmul.
    """
    if "biasT0" in _cache:
        return _cache["biasT0"]
    xs, ys = np.meshgrid(np.arange(GRID), np.arange(GRID), indexing="ij")
    xf = xs.reshape(-1).astype(np.float32)
    yf = ys.reshape(-1).astype(np.float32)
    dist = np.abs(xf[:, None] - xf[None, :]) + np.abs(yf[:, None] - yf[None, :])
    sl0 = -(2.0 ** (-1.0))
    sr0 = -(2.0 ** (-0.5))
    triu = np.triu(np.ones((S, S), np.bool_))  # j >= i
    b0 = np.where(triu, sr0 * dist, sl0 * dist) / SCALE  # [i, j]
    bT = np.ascontiguousarray(b0.T)  # [j, i]
    out = np.ascontiguousarray(
        bT.reshape(S // 128, 128, S).transpose(1, 0, 2)
    ).astype(ml_dtypes.bfloat16)
    _cache["biasT0"] = out
    return out


def _scaled_identities():
    """idh[p, h, m] = 2^-h * eye(128), bf16 (exact powers of two)."""
    if "idh" in _cache:
        return _cache["idh"]
    eye = np.eye(128, dtype=np.float32)
    out = np.empty((128, H, 128), dtype=ml_dtypes.bfloat16)
    for h in range(H):
        out[:, h, :] = (eye * (2.0 ** (-h))).astype(ml_dtypes.bfloat16)
    _cache["idh"] = out
    return out


def _pos_table():
    if "pos" in _cache:
        return _cache["pos"]
    inv_freq = 1.0 / (10000.0 ** (np.arange(0, D, 2, dtype=np.float32) / D))
    t = np.arange(S, dtype=np.float32)
    sinusoid = t[:, None] * inv_freq[None, :]
    _cache["pos"] = np.concatenate(
        [np.sin(sinusoid), np.cos(sinusoid)], axis=-1
    ).astype(np.float32)
    return _cache["pos"]


def _build_bass():
    if "nc" in _cache:
        return _cache["nc"]
    nc = bass.Bass()
    io = {}
    io["x0"] = nc.dram_tensor("x0", [128, NCH, D], F32, kind="ExternalInput")
    io["biasT0"] = nc.dram_tensor("biasT0", [128, S // 128, S], BF16, kind="ExternalInput")
    io["idh"] = nc.dram_tensor("idh", [128, H, 128], BF16, kind="ExternalInput")
    io["w1h"] = nc.dram_tensor("w1h", [128, L, D // 128, H2], F32R, kind="ExternalInput")
    io["b1h"] = nc.dram_tensor("b1h", [128, L, H2 // 128], F32, kind="ExternalInput")
    io["w2h"] = nc.dram_tensor("w2h", [128, L, H2 // 128, D], BF16, kind="ExternalInput")
    io["b2h"] = nc.dram_tensor("b2h", [128, L, D], F32R, kind="ExternalInput")
    io["ln1w"] = nc.dram_tensor("ln1w", [128, L, D], F32, kind="ExternalInput")
    io["ln1b"] = nc.dram_tensor("ln1b", [128, L, D], F32, kind="ExternalInput")
    io["lnfw"] = nc.dram_tensor("lnfw", [128, D], F32, kind="ExternalInput")
    io["lnfb"] = nc.dram_tensor("lnfb", [128, D], F32, kind="ExternalInput")
    y = nc.dram_tensor("y", [128, NCH, D], F32, kind="ExternalOutput")

    with tile.TileContext(nc) as tc, ExitStack() as ctx:
        _emit(ctx, tc, io, y)

    _split_multi_waits(nc)
    _cache["nc"] = nc
    return nc


def _split_multi_waits(nc):
    """walrus codegen on this image only supports ONE sync-wait per TPB
    engine-instruction descriptor. Move excess waits onto sequencer NoOps
    inserted immediately before the instruction (same engine queue)."""
    nsplit = 0
    skip = ("InstNoOp", "InstEventSemaphore")
    for func in nc.m.functions:
        for bb in func.blocks:
            insts = list(bb.instructions)
            out = []
            for inst in insts:
                si = inst.sync_info
                if (si is not None and si.on_wait and len(si.on_wait) > 1
                        and type(inst).__name__ not in skip):
                    for w in list(si.on_wait[:-1]):
                        nop = mybir.InstNoOp(
                            name=f"WSPLIT-{nsplit}", ins=[], outs=[])
                        nop.engine = inst.engine
                        nop.sync_info = mybir.SyncInfo(
                            on_wait=[w], on_update=[])
                        out.append(nop)
                        nsplit += 1
                    si.on_wait = [si.on_wait[-1]]
                out.append(inst)
            if nsplit:
                bb.instructions = out
    return nsplit


def _emit(ctx, tc, io, y):
    nc = tc.nc
    singles = ctx.enter_context(tc.tile_pool(name="singles", bufs=1))
    xp = ctx.enter_context(tc.tile_pool(name="xp", bufs=4))
    sp = ctx.enter_context(tc.tile_pool(name="sp", bufs=4))
    ep = ctx.enter_context(tc.tile_pool(name="ep", bufs=5))
    otp = ctx.enter_context(tc.tile_pool(name="otp", bufs=1))
    htp = ctx.enter_context(tc.tile_pool(name="htp", bufs=2))
    # PSUM (8 banks of 2KB/partition): unit 4 + pv 2 + small 2
    ps_unit = ctx.enter_context(tc.tile_pool(name="ps_unit", bufs=4, space="PSUM"))
    ps_pv = ctx.enter_context(tc.tile_pool(name="ps_pv", bufs=2, space="PSUM"))
    ps_small = ctx.enter_context(tc.tile_pool(name="ps_small", bufs=2, space="PSUM"))

    # ---- resident tensors (all DMA'd once, up front) ----
    # x0 and bias split into per-chunk DMAs so they spread across the DMA
    # queues and the first LN/QK work starts within a few microseconds.
    x_sb = singles.tile([128, NCH, D], F32)
    for c in range(NCH):
        nc.sync.dma_start(out=x_sb[:, c, :], in_=io["x0"][:, c, :])
    bias_sb = singles.tile([128, S // 128, S], BF16)
    for jc in range(S // 128):
        nc.sync.dma_start(out=bias_sb[:, jc, :], in_=io["biasT0"][:, jc, :])
    idh_sb = singles.tile([128, H, 128], BF16)
    nc.sync.dma_start(out=idh_sb, in_=io["idh"][:])
    w1_sb = singles.tile([128, L, D // 128, H2], F32R)
    for l in range(L):
        nc.sync.dma_start(out=w1_sb[:, l, :, :], in_=io["w1h"][:, l, :, :])
    b1_sb = singles.tile([128, L, H2 // 128], F32)
    nc.sync.dma_start(out=b1_sb, in_=io["b1h"][:])
    w2_sb = singles.tile([128, L, H2 // 128, D], BF16)
    nc.sync.dma_start(out=w2_sb, in_=io["w2h"][:])
    b2_sb = singles.tile([128, L, D], F32R)
    nc.sync.dma_start(out=b2_sb, in_=io["b2h"][:])

    id_f32 = singles.tile([128, 128], F32)
    make_identity(nc, id_f32)
    ones_f32 = singles.tile([128, 128], F32)
    nc.vector.memset(ones_f32, 1.0)
    ones128 = singles.tile([128, 128], F32R)
    nc.vector.tensor_copy(out=ones128, in_=ones_f32)
    eps_t = singles.tile([128, 1], F32)
    nc.vector.memset(eps_t, LN_EPS)
    I32 = mybir.dt.int32
    shift1_t = singles.tile([128, 1], I32)
    nc.vector.memset(shift1_t, 1)
    magic_t = singles.tile([128, 16], I32)
    nc.vector.memset(magic_t, 0x5F3759DF)
    absorb_scratch = singles.tile([128, 16], F32)
    absorb_n = [0]

    def absorb(ap):
        # DVE wait absorber: DVE-struct instructions support only one sync
        # wait on this codegen, so soak the DMA-completion wait into a copy.
        k = absorb_n[0] % 16
        absorb_n[0] += 1
        nc.vector.tensor_copy(out=absorb_scratch[:, k:k + 1],
                              in_=ap[0:128, 0:1])

    ln1w_sb = singles.tile([128, L, D], F32)
    nc.sync.dma_start(out=ln1w_sb, in_=io["ln1w"][:])
    absorb(ln1w_sb[:, 0, :])
    ln1b_sb = singles.tile([128, L, D], F32)
    nc.sync.dma_start(out=ln1b_sb, in_=io["ln1b"][:])
    absorb(ln1b_sb[:, 0, :])
    lnfw_sb = singles.tile([128, D], F32)
    nc.sync.dma_start(out=lnfw_sb, in_=io["lnfw"][:])
    absorb(lnfw_sb)
    lnfb_sb = singles.tile([128, D], F32)
    nc.sync.dma_start(out=lnfb_sb, in_=io["lnfb"][:])
    absorb(lnfb_sb)

    v_aug = singles.tile([128, NCH, H, 66], BF16)
    nc.vector.memset(v_aug, 1.0)
    # xnT: [half][128, T] transposed layernormed activations (f32r)
    xnT = []
    for i in range(2):
        xnT_half = singles.tile([128, T], F32R, tag=f"xnT{i}")
        xnT.append(xnT_half)
    # kz[:, h%4, :]: zero-banded K-side stationary for the current head; only
    # rows [hp, hp+32) are live so the QK matmul contracts K=128 (full-tile
    # PE config) with the other heads' rows multiplied by zero. Slot = band
    # position, so the zeros outside each band are written exactly once.
    kz = singles.tile([128, 4, S], F32R)
    zero_t = singles.tile([128, 512], BF16)
    nc.vector.memset(zero_t, 0.0)
    for zi in range(4 * S // 512):
        nc.vector.tensor_copy(
            out=kz.rearrange("p a b -> p (a b)")[:, zi * 512:(zi + 1) * 512],
            in_=zero_t,
        )
    EXP = mybir.ActivationFunctionType.Exp

    def transpose_to(xn, c):
        for half in range(2):
            pt = ps_small.tile([128, 128], F32, tag="small", name="pt")
            nc.tensor.transpose(pt, xn[:, half * 128:(half + 1) * 128], id_f32)
            nc.vector.tensor_copy(
                out=xnT[half][:, c * 128:(c + 1) * 128], in_=pt
            )

    def build_kz(b, h):
        hp = (h % 4) * HD
        nc.gpsimd.tensor_copy(
            out=kz[hp:hp + HD, h % 4, :],
            in_=xnT[h // 4][hp:hp + HD, b * S:(b + 1) * S],
        )

    def ln_stats(chunks):
        """bn stats + rsqrt for a chunk list; returns (mv_all, rs_all)."""
        n = len(chunks)
        mv_all = sp.tile([128, n, 2], F32, tag="mv")
        rs_all = sp.tile([128, n], F32, tag="rs")
        for k, c in enumerate(chunks):
            st = sp.tile([128, 6], F32, tag="st")
            nc.vector.bn_stats(out=st, in_=x_sb[:, c, :])
            nc.vector.bn_aggr(out=mv_all[:, k, :], in_=st)
        # rsqrt(var+eps) entirely on DVE (quake seed + 3 Newton steps) --
        # keeps Ln/Exp out of the scalar engine so activation tables never
        # thrash against Gelu/Exp phases.
        t_all = sp.tile([128, n], F32, tag="t")
        nc.vector.tensor_scalar_add(t_all, mv_all[:, :, 1], eps_t)
        nc.vector.tensor_scalar(
            out=rs_all.bitcast(I32), in0=t_all.bitcast(I32),
            scalar1=shift1_t, scalar2=None,
            op0=mybir.AluOpType.logical_shift_right,
        )
        nc.vector.tensor_tensor(
            out=rs_all.bitcast(I32), in0=magic_t[:, 0:n],
            in1=rs_all.bitcast(I32), op=mybir.AluOpType.subtract,
        )
        h_all = sp.tile([128, n], F32, tag="h")
        for _ in range(3):
            nc.vector.tensor_mul(out=h_all, in0=rs_all, in1=rs_all)
            nc.vector.tensor_mul(out=h_all, in0=h_all, in1=t_all)
            nc.vector.tensor_scalar(
                out=h_all, in0=h_all, scalar1=-0.5, scalar2=1.5,
                op0=mybir.AluOpType.mult, op1=mybir.AluOpType.add,
            )
            nc.vector.tensor_mul(out=rs_all, in0=rs_all, in1=h_all)
        return mv_all, rs_all

    def ln_apply(k, c, mv_all, rs_all, affine, out_cb, inplace_into=None):
        if inplace_into is not None:
            xn = inplace_into(c)
        else:
            xn = xp.tile([128, D], F32, tag="xn")
        nc.vector.tensor_scalar(
            out=xn, in0=x_sb[:, c, :],
            scalar1=mv_all[:, k, 0:1], scalar2=rs_all[:, k:k + 1],
            op0=mybir.AluOpType.subtract, op1=mybir.AluOpType.mult,
        )
        if affine is not None:
            w_ap, b_ap = affine
            nc.vector.tensor_mul(out=xn, in0=xn, in1=w_ap)
            nc.vector.tensor_add(out=xn, in0=xn, in1=b_ap)
        if out_cb is not None:
            out_cb(c, xn)

    def layer_norm_chunks(chunks, affine, out_cb, inplace_into=None):
        mv_all, rs_all = ln_stats(chunks)
        for k, c in enumerate(chunks):
            ln_apply(k, c, mv_all, rs_all, affine, out_cb, inplace_into)

    def ln_hook(bi, affine, out_cb, inplace_into=None):
        """Returns a 0-arg hook emitting stats on the first call, then one
        LN chunk per call; further calls are no-ops."""
        chunks = list(range(bi * 8, bi * 8 + 8))
        state = {"step": 0}

        def hook():
            step = state["step"]
            state["step"] += 1
            if step == 0:
                state["mv"], state["rs"] = ln_stats(chunks)
            elif step <= len(chunks):
                k = step - 1
                ln_apply(k, chunks[k], state["mv"], state["rs"],
                         affine, out_cb, inplace_into)
        return hook

    def a_cb(c, xn):
        nc.gpsimd.tensor_copy(
            out=v_aug[:, c, :, 0:HD],
            in_=xn.rearrange("p (h d) -> p h d", h=H),
        )
        transpose_to(xn, c)

    def c_cb(c, xn):
        transpose_to(xn, c)

    def phase_a_hook(bi, l):
        return ln_hook(bi, (ln1w_sb[:, l, :], ln1b_sb[:, l, :]), a_cb)

    def phase_c_hook(bi, l):
        return ln_hook(bi, None, c_cb)

    kz_ready = [None]  # batch whose first 3 kz bands are already built
    ot_state = [None]  # ot2 buffer of the currently-draining head pair

    # pending attention-output epilogue, carried ACROSS phases and flushed
    # one 128-token block per pipeline step so its PE transposes always have
    # matmuls to hide between.
    pending = [None]

    def flush_step():
        # one 128-token block for a PAIR of heads packed in ot2:
        # rows 0..32 = head hb (dims+den), rows 33..65 = head hb+1
        if pending[0] is None:
            return
        b, hb, ot2, ic = pending[0]
        ptt = ps_small.tile([128, 97], F32, tag="small", name="ptt")
        nc.tensor.transpose(
            ptt, ot2[:, ic * 128:(ic + 1) * 128], id_f32[0:97, 0:97]
        )
        c = b * 8 + ic
        for g in range(2):
            rt = sp.tile([128, 1], F32, tag="rt")
            nc.vector.reciprocal(out=rt, in_=ptt[:, g * 64 + 32:g * 64 + 33])
            xs = x_sb[:, c, (hb + g) * HD:(hb + g + 1) * HD]
            nc.vector.scalar_tensor_tensor(
                out=xs, in0=ptt[:, g * 64:g * 64 + HD], scalar=rt, in1=xs,
                op0=mybir.AluOpType.mult, op1=mybir.AluOpType.add,
            )
        pending[0] = (b, hb, ot2, ic + 1) if ic + 1 < 8 else None

    def phase_b(b, hook=None, prefetch=None):
        """Attention for batch b, all heads; two-deep unit pipeline.
        hook() interleaves another phase's LN emission; first call comes
        after the spread flush so hooked stats see the final x. prefetch
        names the batch whose first kz bands to build during our tail."""
        hook_u = 10
        if kz_ready[0] != b:
            for h in range(3):
                build_kz(b, h)

        for h in range(H):
            if h + 3 < H:
                build_kz(b, h + 3)
            elif prefetch is not None:
                build_kz(prefetch, h - 5)
                kz_ready[0] = prefetch
            xnT_h = xnT[h // 4]
            po = [ps_pv.tile([65, 512], F32, tag="pv", name=f"po{it}")
                  for it in range(2)]
            units = [(jc, it) for jc in range(S // 128) for it in range(2)]
            ps_q = {}

            def qkb(u, h=h, xnT_h=xnT_h, ps_q=ps_q, units=units):
                jc, it = units[u]
                ps = ps_unit.tile([128, 512], F32, tag="unit", name="ps")
                qtile = xnT_h[:, b * S + it * 512: b * S + (it + 1) * 512]
                nc.tensor.matmul(
                    ps, lhsT=kz[:, h % 4, jc * 128:(jc + 1) * 128], rhs=qtile,
                    start=True, stop=False,
                )
                nc.tensor.matmul(
                    ps, lhsT=idh_sb[:, h, :],
                    rhs=bias_sb[:, jc, it * 512:(it + 1) * 512],
                    start=False, stop=True,
                )
                ps_q[u] = ps

            qkb(0)
            qkb(1)
            for u in range(16):
                jc, it = units[u]
                et = ep.tile([128, 512], BF16, tag="et")
                nc.scalar.activation(out=et, in_=ps_q[u], func=EXP, scale=SCALE)
                if u + 2 < 16:
                    qkb(u + 2)
                if 2 <= u <= 9:
                    flush_step()
                if u == hook_u and hook is not None:
                    hook()
                nc.tensor.matmul(
                    po[it], lhsT=v_aug[:, b * 8 + jc, h, 0:65], rhs=et,
                    start=(jc == 0), stop=(jc == S // 128 - 1),
                )
            # drain PV psum -> SBUF, packing head pairs into one ot2;
            # PE transposes deferred into later pipeline steps
            if h % 2 == 0:
                ot_state[0] = otp.tile([97, S], F32, tag="ot", name="ot2")
            ot2 = ot_state[0]
            r0 = (h % 2) * 64
            for it in range(2):
                nc.vector.tensor_copy(
                    out=ot2[r0:r0 + 33, it * 512:(it + 1) * 512],
                    in_=po[it][0:33, :],
                )
            if h % 2 == 1:
                pending[0] = (b, h - 1, ot2, 0)
            if hook is not None:
                hook()

    def phase_d(bi, l, hook=None, prefetch=None):
        """MLP for batch bi (tokens bi*1024 .. +1024). hook() interleaves
        another phase's LN; first call comes after the spread flush."""
        can_hook = pending[0] is None
        last_tt = bi * 2 + 1
        for tt in range(bi * 2, bi * 2 + 2):
            hT = htp.tile([128, H2 // 128, 512], BF16, tag="hT")
            for hb in range(H2 // 128):
                flush_step()
                if can_hook and hook is not None:
                    hook()
                pm = ps_unit.tile([128, 512], F32, tag="unit", name="pm")
                for k in range(D // 128):
                    nc.tensor.matmul(
                        pm,
                        lhsT=w1_sb[:, l, k, hb * 128:(hb + 1) * 128],
                        rhs=xnT[k][:, tt * 512:(tt + 1) * 512],
                        start=(k == 0), stop=(k == D // 128 - 1),
                    )
                nc.scalar.activation(
                    out=hT[:, hb, :], in_=pm,
                    func=mybir.ActivationFunctionType.Gelu,
                    bias=b1_sb[:, l, hb:hb + 1],
                )
            can_hook = True  # pending fully flushed by first tt's hb loop
            if tt == last_tt and prefetch is not None:
                for h in range(3):
                    build_kz(prefetch, h)
                kz_ready[0] = prefetch
            for t2 in range(4):
                if hook is not None:
                    hook()
                pm2 = ps_small.tile([128, D], F32, tag="small", name="pm2")
                for hb in range(H2 // 128):
                    nc.tensor.matmul(
                        pm2,
                        lhsT=hT[:, hb, t2 * 128:(t2 + 1) * 128],
                        rhs=w2_sb[:, l, hb, :],
                        start=(hb == 0), stop=False,
                    )
                nc.tensor.matmul(
                    pm2, lhsT=ones128, rhs=b2_sb[:, l, :],
                    start=False, stop=True,
                )
                c = tt * 4 + t2
                nc.vector.tensor_add(
                    out=x_sb[:, c, :], in0=x_sb[:, c, :], in1=pm2
                )

    def lnf_cb(c, xn):
        nc.sync.dma_start(out=y[:, c, :], in_=xn)

    # ---- main schedule: batch-interleaved phases ----
    # Each LN phase is emitted chunk-wise via hooks inside the preceding
    # PE-heavy phase so its vector/scalar latency hides under matmuls.
    layer_norm_chunks(range(0, 8), (ln1w_sb[:, 0, :], ln1b_sb[:, 0, :]), a_cb)
    for l in range(L):
        phase_b(0, hook=phase_a_hook(1, l), prefetch=1)  # LN1(b1) in B(b0)
        phase_b(1, hook=phase_c_hook(0, l))   # LN2(b0) inside B(b1)
        phase_d(0, l, hook=phase_c_hook(1, l))  # LN2(b1) inside D(b0)
        if l + 1 < L:
            # LN1(b0,l+1) inside D(b1); prefetch next layer's kz tail-end
            phase_d(1, l, hook=phase_a_hook(0, l + 1), prefetch=0)
        else:
            # final LN of batch 0 (in place, per-chunk output DMA) in D(b1)
            phase_d(1, l, hook=ln_hook(0, (lnfw_sb, lnfb_sb), lnf_cb,
                                       inplace_into=lambda c: x_sb[:, c, :]))

    # ---- final LN for batch 1 (tail); in place, per-chunk DMA out ----
    layer_norm_chunks(range(8, NCH), (lnfw_sb, lnfb_sb), lnf_cb,
                      inplace_into=lambda c: x_sb[:, c, :])


def _install_ntff_hook():
    """Wire antenv.axon_hooks NTFF profiling via libaxon ctypes (dev only)."""
    if _cache.get("hook_done"):
        return
    _cache["hook_done"] = True
    try:
        import types
        import sys
        try:
            from antenv.axon_hooks import set_axon_ntff_profile_hook  # noqa
        except ImportError:
            import antenv
            mod = types.ModuleType("antenv.axon_hooks")
            holder = [None]
            mod.set_axon_ntff_profile_hook = lambda h: holder.__setitem__(0, h)
            mod.get_axon_ntff_profile_hook = lambda: holder[0]
            sys.modules["antenv.axon_hooks"] = mod
            antenv.axon_hooks = mod
            from trn_agent_boot.trn_boot import _ntff_profile_via_ctypes
            mod.set_axon_ntff_profile_hook(
                _ntff_profile_via_ctypes("/opt/axon/libaxon_pjrt.so"))
    except Exception as e:  # fail-soft: tracing degrades, run still works
        print("ntff hook install failed:", e)


def kernel(tokens, pos_ids, emb_table, input_weight, position_weight,
           ln1_w, ln1_b, ln2_w, ln2_b, w1, b1, w2, b2, lnf_w, lnf_b):
    tokens = np.asarray(tokens)
    pos_ids = np.asarray(pos_ids)
    emb_table = np.asarray(emb_table, dtype=np.float32)
    x0 = (np.float32(np.asarray(input_weight).reshape(-1)[0])
          * emb_table[tokens]
          + np.float32(np.asarray(position_weight).reshape(-1)[0])
          * _pos_table()[np.asarray(pos_ids)][None]).astype(np.float32)

    w1 = np.asarray(w1, np.float32)
    b1 = np.asarray(b1, np.float32)
    w2 = np.asarray(w2, np.float32)
    b2 = np.asarray(b2, np.float32)
    ln2_w = np.asarray(ln2_w, np.float32)
    ln2_b = np.asarray(ln2_b, np.float32)
    # fold LN2 affine into MLP weights
    w1eff = ln2_w[:, :, None] * w1                     # [L, D, H2]
    b1eff = b1 + np.einsum("ld,ldh->lh", ln2_b, w1)    # [L, H2]
    w1h = np.ascontiguousarray(
        w1eff.reshape(L, D // 128, 128, H2).transpose(2, 0, 1, 3))
    b1h = np.ascontiguousarray(
        b1eff.reshape(L, H2 // 128, 128).transpose(2, 0, 1))
    w2h = np.ascontiguousarray(
        w2.reshape(L, H2 // 128, 128, D).transpose(2, 0, 1, 3)
    ).astype(ml_dtypes.bfloat16)

    nc = _build_bass()
    base = {
        "biasT0": _alibi_bias0T(),
        "idh": _scaled_identities(),
        "w1h": w1h,
        "b1h": b1h,
        "w2h": w2h,
        "b2h": np.ascontiguousarray(np.broadcast_to(
            (b2 / 128.0)[None, :, :], (128, L, D))).astype(np.float32),
        "ln1w": np.ascontiguousarray(np.broadcast_to(
            np.asarray(ln1_w, np.float32)[None, :, :], (128, L, D))),
        "ln1b": np.ascontiguousarray(np.broadcast_to(
            np.asarray(ln1_b, np.float32)[None, :, :], (128, L, D))),
        "lnfw": np.ascontiguousarray(np.broadcast_to(
            np.asarray(lnf_w, np.float32)[None, :], (128, D))),
        "lnfb": np.ascontiguousarray(np.broadcast_to(
            np.asarray(lnf_b, np.float32)[None, :], (128, D))),
    }
    in_maps = []
    for core in range(NC):
        xc = x0[core * BL:(core + 1) * BL].reshape(T, D)
        xh = np.ascontiguousarray(
            xc.reshape(NCH, 128, D).transpose(1, 0, 2))
        m = dict(base)
        m["x0"] = xh
        in_maps.append(m)

    trace = os.environ.get("KERNEL_TRACE", "0") == "1"
    if trace:
        _install_ntff_hook()
    res = run_bass_kernel_spmd(
        nc, in_maps, core_ids=list(range(NC)), trace=trace,
        trace_cores=[0] if trace else None,
    )
    if trace and res.exec_time_ns is not None:
        print(f"HW exec time: {res.exec_time_ns} ns")
        if res.instructions_and_trace is not None:
            print("trace:", res.instructions_and_trace[1])

    out = np.empty((B, S, D), np.float32)
    for core in range(NC):
        yh = res.results[core]["y"]  # [128, NCH, D]
        yc = yh.transpose(1, 0, 2).reshape(BL, S, D)
        out[core * BL:(core + 1) * BL] = yc
    return out


# revision 68
# speedup vs baseline: 1.0291x; 1.0291x over previous
"""Trainium2 Bass kernel for nn_EnhancedStateEncoder (6-layer dense transformer).

Strategy: data-parallel over batch across 8 NeuronCores (2 batches/core).
 - Embedding + sinusoidal pos-emb folded on host (cheap gather).
 - Alibi bias is rank-1 across heads (slopes are exact powers of two:
   bias[h] = 2^-h * bias[0]), so only bias[0]^T is stored (bf16, prescaled
   by 1/sqrt(d)); it is added to Q@K^T PSUM via a per-head scaled-identity
   matmul on the TensorEngine.
 - QK^T and the first MLP matmul stream as float32r (tf32-like, 1 cycle/row
   for moving dim >= 256 vs 4 for fp32).
 - Attention computed in S^T layout [j(part), i(free)] in [128,512] PSUM
   units, software-pipelined two units deep so the PE never waits for the
   ScalarE exp; a ones-column appended to V yields softmax denominators.
 - Per-head output epilogue (PSUM drain, transpose, 1/denom scale) is
   deferred into the next head's unit loop to avoid PE stalls.
 - LayerNorm via bn_stats/bn_aggr; rsqrt as exp(-0.5*ln(var+eps)).
 - LN2's affine is folded into the MLP's first matmul on the host.
 - Phases alternate between the two batches per core so LN (vector/scalar)
   work of one batch hides under attention/MLP (PE) work of the other.
 - All layer weights are DMA'd into SBUF once, up front.
"""

import math
import os
from contextlib import ExitStack

import numpy as np
import ml_dtypes

import concourse.bass as bass
import concourse.mybir as mybir
import concourse.tile as tile
from concourse.bass_utils import run_bass_kernel_spmd
from concourse.masks import make_identity

F32 = mybir.dt.float32
F32R = mybir.dt.float32r
BF16 = mybir.dt.bfloat16

B, S, D, H, HD, L, H2 = 16, 1024, 256, 8, 32, 6, 1024
NC = 8            # cores
BL = B // NC      # batches per core = 2
T = BL * S        # tokens per core = 2048
NCH = T // 128    # 128-token chunks per core = 16
SCALE = 1.0 / math.sqrt(HD)
LN_EPS = 1e-5
GRID = 32

_cache = {}


def _alibi_bias0T():
    """biasT0[p, jc, i] = bias[h=0, i, jc*128+p] / SCALE, bf16.

    bias[h] == 2^-h * bias[0] exactly (slopes are powers of two), so only
    head 0 is stored; per-head scaling happens in the identity matmul.
    """
    if "biasT0" in _cache:
        return _cache["biasT0"]
    xs, ys = np.meshgrid(np.arange(GRID), np.arange(GRID), indexing="ij")
    xf = xs.reshape(-1).astype(np.float32)
    yf = ys.reshape(-1).astype(np.float32)
    dist = np.abs(xf[:, None] - xf[None, :]) + np.abs(yf[:, None] - yf[None, :])
    sl0 = -(2.0 ** (-1.0))
    sr0 = -(2.0 ** (-0.5))
    triu = np.triu(np.ones((S, S), np.bool_))  # j >= i
    b0 = np.where(triu, sr0 * dist, sl0 * dist) / SCALE  # [i, j]
    bT = np.ascontiguousarray(b0.T)  # [j, i]
    out = np.ascontiguousarray(
        bT.reshape(S // 128, 128, S).transpose(1, 0, 2)
    ).astype(ml_dtypes.bfloat16)
    _cache["biasT0"] = out
    return out


def _scaled_identities():
    """idh[p, h, m] = 2^-h * eye(128), bf16 (exact powers of two)."""
    if "idh" in _cache:
        return _cache["idh"]
    eye = np.eye(128, dtype=np.float32)
    out = np.empty((128, H, 128), dtype=ml_dtypes.bfloat16)
    for h in range(H):
        out[:, h, :] = (eye * (2.0 ** (-h))).astype(ml_dtypes.bfloat16)
    _cache["idh"] = out
    return out


def _pos_table():
    if "pos" in _cache:
        return _cache["pos"]
    inv_freq = 1.0 / (10000.0 ** (np.arange(0, D, 2, dtype=np.float32) / D))
    t = np.arange(S, dtype=np.float32)
    sinusoid = t[:, None] * inv_freq[None, :]
    _cache["pos"] = np.concatenate(
        [np.sin(sinusoid), np.cos(sinusoid)], axis=-1
    ).astype(np.float32)
    return _cache["pos"]


def _build_bass():
    if "nc" in _cache:
        return _cache["nc"]
    nc = bass.Bass()
    io = {}
    io["x0"] = nc.dram_tensor("x0", [128, NCH, D], F32, kind="ExternalInput")
    io["biasT0"] = nc.dram_tensor("biasT0", [128, S // 128, S], BF16, kind="ExternalInput")
    io["idh"] = nc.dram_tensor("idh", [128, H, 128], BF16, kind="ExternalInput")
    io["w1h"] = nc.dram_tensor("w1h", [128, L, D // 128, H2], F32R, kind="ExternalInput")
    io["b1h"] = nc.dram_tensor("b1h", [128, L, H2 // 128], F32, kind="ExternalInput")
    io["w2h"] = nc.dram_tensor("w2h", [128, L, H2 // 128, D], BF16, kind="ExternalInput")
    io["b2h"] = nc.dram_tensor("b2h", [128, L, D], F32R, kind="ExternalInput")
    io["ln1w"] = nc.dram_tensor("ln1w", [128, L, D], F32, kind="ExternalInput")
    io["ln1b"] = nc.dram_tensor("ln1b", [128, L, D], F32, kind="ExternalInput")
    io["lnfw"] = nc.dram_tensor("lnfw", [128, D], F32, kind="ExternalInput")
    io["lnfb"] = nc.dram_tensor("lnfb", [128, D], F32, kind="ExternalInput")
    y = nc.dram_tensor("y", [128, NCH, D], F32, kind="ExternalOutput")

    with tile.TileContext(nc) as tc, ExitStack() as ctx:
        _emit(ctx, tc, io, y)

    _split_multi_waits(nc)
    _cache["nc"] = nc
    return nc


def _split_multi_waits(nc):
    """walrus codegen on this image only supports ONE sync-wait per TPB
    engine-instruction descriptor. Move excess waits onto sequencer NoOps
    inserted immediately before the instruction (same engine queue)."""
    nsplit = 0
    skip = ("InstNoOp", "InstEventSemaphore")
    for func in nc.m.functions:
        for bb in func.blocks:
            insts = list(bb.instructions)
            out = []
            for inst in insts:
                si = inst.sync_info
                if (si is not None and si.on_wait and len(si.on_wait) > 1
                        and type(inst).__name__ not in skip):
                    for w in list(si.on_wait[:-1]):
                        nop = mybir.InstNoOp(
                            name=f"WSPLIT-{nsplit}", ins=[], outs=[])
                        nop.engine = inst.engine
                        nop.sync_info = mybir.SyncInfo(
                            on_wait=[w], on_update=[])
                        out.append(nop)
                        nsplit += 1
                    si.on_wait = [si.on_wait[-1]]
                out.append(inst)
            if nsplit:
                bb.instructions = out
    return nsplit


def _emit(ctx, tc, io, y):
    nc = tc.nc
    singles = ctx.enter_context(tc.tile_pool(name="singles", bufs=1))
    xp = ctx.enter_context(tc.tile_pool(name="xp", bufs=4))
    sp = ctx.enter_context(tc.tile_pool(name="sp", bufs=4))
    ep = ctx.enter_context(tc.tile_pool(name="ep", bufs=5))
    otp = ctx.enter_context(tc.tile_pool(name="otp", bufs=1))
    htp = ctx.enter_context(tc.tile_pool(name="htp", bufs=2))
    # PSUM (8 banks of 2KB/partition): unit 4 + pv 2 + small 2
    ps_unit = ctx.enter_context(tc.tile_pool(name="ps_unit", bufs=4, space="PSUM"))
    ps_pv = ctx.enter_context(tc.tile_pool(name="ps_pv", bufs=2, space="PSUM"))
    ps_small = ctx.enter_context(tc.tile_pool(name="ps_small", bufs=2, space="PSUM"))

    # ---- resident tensors (all DMA'd once, up front) ----
    # x0 and bias split into per-chunk DMAs so they spread across the DMA
    # queues and the first LN/QK work starts within a few microseconds.
    x_sb = singles.tile([128, NCH, D], F32)
    for c in range(NCH):
        nc.sync.dma_start(out=x_sb[:, c, :], in_=io["x0"][:, c, :])
    bias_sb = singles.tile([128, S // 128, S], BF16)
    for jc in range(S // 128):
        nc.sync.dma_start(out=bias_sb[:, jc, :], in_=io["biasT0"][:, jc, :])
    idh_sb = singles.tile([128, H, 128], BF16)
    nc.sync.dma_start(out=idh_sb, in_=io["idh"][:])
    w1_sb = singles.tile([128, L, D // 128, H2], F32R)
    for l in range(L):
        nc.sync.dma_start(out=w1_sb[:, l, :, :], in_=io["w1h"][:, l, :, :])
    b1_sb = singles.tile([128, L, H2 // 128], F32)
    nc.sync.dma_start(out=b1_sb, in_=io["b1h"][:])
    w2_sb = singles.tile([128, L, H2 // 128, D], BF16)
    nc.sync.dma_start(out=w2_sb, in_=io["w2h"][:])
    b2_sb = singles.tile([128, L, D], F32R)
    nc.sync.dma_start(out=b2_sb, in_=io["b2h"][:])

    id_f32 = singles.tile([128, 128], F32)
    make_identity(nc, id_f32)
    ones_f32 = singles.tile([128, 128], F32)
    nc.vector.memset(ones_f32, 1.0)
    ones128 = singles.tile([128, 128], F32R)
    nc.vector.tensor_copy(out=ones128, in_=ones_f32)
    eps_t = singles.tile([128, 1], F32)
    nc.vector.memset(eps_t, LN_EPS)
    I32 = mybir.dt.int32
    shift1_t = singles.tile([128, 1], I32)
    nc.vector.memset(shift1_t, 1)
    magic_t = singles.tile([128, 16], I32)
    nc.vector.memset(magic_t, 0x5F3759DF)
    absorb_scratch = singles.tile([128, 16], F32)
    absorb_n = [0]

    def absorb(ap):
        # DVE wait absorber: DVE-struct instructions support only one sync
        # wait on this codegen, so soak the DMA-completion wait into a copy.
        k = absorb_n[0] % 16
        absorb_n[0] += 1
        nc.vector.tensor_copy(out=absorb_scratch[:, k:k + 1],
                              in_=ap[0:128, 0:1])

    ln1w_sb = singles.tile([128, L, D], F32)
    nc.sync.dma_start(out=ln1w_sb, in_=io["ln1w"][:])
    absorb(ln1w_sb[:, 0, :])
    ln1b_sb = singles.tile([128, L, D], F32)
    nc.sync.dma_start(out=ln1b_sb, in_=io["ln1b"][:])
    absorb(ln1b_sb[:, 0, :])
    lnfw_sb = singles.tile([128, D], F32)
    nc.sync.dma_start(out=lnfw_sb, in_=io["lnfw"][:])
    absorb(lnfw_sb)
    lnfb_sb = singles.tile([128, D], F32)
    nc.sync.dma_start(out=lnfb_sb, in_=io["lnfb"][:])
    absorb(lnfb_sb)

    v_aug = singles.tile([128, NCH, H, 66], BF16)
    nc.vector.memset(v_aug, 1.0)
    # xnT: [half][128, T] transposed layernormed activations (f32r)
    xnT = []
    for i in range(2):
        xnT_half = singles.tile([128, T], F32R, tag=f"xnT{i}")
        xnT.append(xnT_half)
    # kz[:, h%4, :]: zero-banded K-side stationary for the current head; only
    # rows [hp, hp+32) are live so the QK matmul contracts K=128 (full-tile
    # PE config) with the other heads' rows multiplied by zero. Slot = band
    # position, so the zeros outside each band are written exactly once.
    kz = singles.tile([128, 4, S], F32R)
    zero_t = singles.tile([128, 512], BF16)
    nc.vector.memset(zero_t, 0.0)
    for zi in range(4 * S // 512):
        nc.vector.tensor_copy(
            out=kz.rearrange("p a b -> p (a b)")[:, zi * 512:(zi + 1) * 512],
            in_=zero_t,
        )
    EXP = mybir.ActivationFunctionType.Exp

    def transpose_to(xn, c):
        for half in range(2):
            pt = ps_small.tile([128, 128], F32, tag="small", name="pt")
            nc.tensor.transpose(pt, xn[:, half * 128:(half + 1) * 128], id_f32)
            nc.vector.tensor_copy(
                out=xnT[half][:, c * 128:(c + 1) * 128], in_=pt
            )

    def build_kz(b, h):
        hp = (h % 4) * HD
        nc.gpsimd.tensor_copy(
            out=kz[hp:hp + HD, h % 4, :],
            in_=xnT[h // 4][hp:hp + HD, b * S:(b + 1) * S],
        )

    def ln_stats(chunks):
        """bn stats + rsqrt for a chunk list; returns (mv_all, rs_all)."""
        n = len(chunks)
        mv_all = sp.tile([128, n, 2], F32, tag="mv")
        rs_all = sp.tile([128, n], F32, tag="rs")
        for k, c in enumerate(chunks):
            st = sp.tile([128, 6], F32, tag="st")
            nc.vector.bn_stats(out=st, in_=x_sb[:, c, :])
            nc.vector.bn_aggr(out=mv_all[:, k, :], in_=st)
        # rsqrt(var+eps) entirely on DVE (quake seed + 3 Newton steps) --
        # keeps Ln/Exp out of the scalar engine so activation tables never
        # thrash against Gelu/Exp phases.
        t_all = sp.tile([128, n], F32, tag="t")
        nc.vector.tensor_scalar_add(t_all, mv_all[:, :, 1], eps_t)
        nc.vector.tensor_scalar(
            out=rs_all.bitcast(I32), in0=t_all.bitcast(I32),
            scalar1=shift1_t, scalar2=None,
            op0=mybir.AluOpType.logical_shift_right,
        )
        nc.vector.tensor_tensor(
            out=rs_all.bitcast(I32), in0=magic_t[:, 0:n],
            in1=rs_all.bitcast(I32), op=mybir.AluOpType.subtract,
        )
        h_all = sp.tile([128, n], F32, tag="h")
        for _ in range(3):
            nc.vector.tensor_mul(out=h_all, in0=rs_all, in1=rs_all)
            nc.vector.tensor_mul(out=h_all, in0=h_all, in1=t_all)
            nc.vector.tensor_scalar(
                out=h_all, in0=h_all, scalar1=-0.5, scalar2=1.5,
                op0=mybir.AluOpType.mult, op1=mybir.AluOpType.add,
            )
            nc.vector.tensor_mul(out=rs_all, in0=rs_all, in1=h_all)
        return mv_all, rs_all

    def ln_apply(k, c, mv_all, rs_all, affine, out_cb, inplace_into=None):
        if inplace_into is not None:
            xn = inplace_into(c)
        else:
            xn = xp.tile([128, D], F32, tag="xn")
        nc.vector.tensor_scalar(
            out=xn, in0=x_sb[:, c, :],
            scalar1=mv_all[:, k, 0:1], scalar2=rs_all[:, k:k + 1],
            op0=mybir.AluOpType.subtract, op1=mybir.AluOpType.mult,
        )
        if affine is not None:
            w_ap, b_ap = affine
            nc.vector.tensor_mul(out=xn, in0=xn, in1=w_ap)
            nc.vector.tensor_add(out=xn, in0=xn, in1=b_ap)
        if out_cb is not None:
            out_cb(c, xn)

    def layer_norm_chunks(chunks, affine, out_cb, inplace_into=None):
        mv_all, rs_all = ln_stats(chunks)
        for k, c in enumerate(chunks):
            ln_apply(k, c, mv_all, rs_all, affine, out_cb, inplace_into)

    def ln_hook(bi, affine, out_cb, inplace_into=None):
        """Returns a 0-arg hook emitting stats on the first call, then one
        LN chunk per call; further calls are no-ops."""
        chunks = list(range(bi * 8, bi * 8 + 8))
        state = {"step": 0}

        def hook():
            step = state["step"]
            state["step"] += 1
            if step == 0:
                state["mv"], state["rs"] = ln_stats(chunks)
            elif step <= len(chunks):
                k = step - 1
                ln_apply(k, chunks[k], state["mv"], state["rs"],
                         affine, out_cb, inplace_into)
        return hook

    def a_cb(c, xn):
        nc.gpsimd.tensor_copy(
            out=v_aug[:, c, :, 0:HD],
            in_=xn.rearrange("p (h d) -> p h d", h=H),
        )
        transpose_to(xn, c)

    def c_cb(c, xn):
        transpose_to(xn, c)

    def phase_a_hook(bi, l):
        return ln_hook(bi, (ln1w_sb[:, l, :], ln1b_sb[:, l, :]), a_cb)

    def phase_c_hook(bi, l):
        return ln_hook(bi, None, c_cb)

    kz_ready = [None]  # batch whose first 3 kz bands are already built

    # pending attention-output epilogue, carried ACROSS phases and flushed
    # one 128-token block per pipeline step so its PE transposes always have
    # matmuls to hide between.
    pending = [None]

    def flush_step():
        if pending[0] is None:
            return
        b, h, ot, ic = pending[0]
        ptt = ps_small.tile([128, 65], F32, tag="small", name="ptt")
        nc.tensor.transpose(
            ptt, ot[:, ic * 128:(ic + 1) * 128], id_f32[0:65, 0:65]
        )
        rt = sp.tile([128, 1], F32, tag="rt")
        nc.vector.reciprocal(out=rt, in_=ptt[:, 32:33])
        c = b * 8 + ic
        xs = x_sb[:, c, h * HD:(h + 1) * HD]
        nc.vector.scalar_tensor_tensor(
            out=xs, in0=ptt[:, 0:HD], scalar=rt, in1=xs,
            op0=mybir.AluOpType.mult, op1=mybir.AluOpType.add,
        )
        pending[0] = (b, h, ot, ic + 1) if ic + 1 < 8 else None

    def phase_b(b, hook=None, prefetch=None):
        """Attention for batch b, all heads; two-deep unit pipeline.
        hook() interleaves another phase's LN emission; first call comes
        after the spread flush so hooked stats see the final x. prefetch
        names the batch whose first kz bands to build during our tail."""
        hook_u = 10
        if kz_ready[0] != b:
            for h in range(3):
                build_kz(b, h)

        for h in range(H):
            if h + 3 < H:
                build_kz(b, h + 3)
            elif prefetch is not None:
                build_kz(prefetch, h - 5)
                kz_ready[0] = prefetch
            xnT_h = xnT[h // 4]
            po = [ps_pv.tile([65, 512], F32, tag="pv", name=f"po{it}")
                  for it in range(2)]
            units = [(jc, it) for jc in range(S // 128) for it in range(2)]
            ps_q = {}

            def qkb(u, h=h, xnT_h=xnT_h, ps_q=ps_q, units=units):
                jc, it = units[u]
                ps = ps_unit.tile([128, 512], F32, tag="unit", name="ps")
                qtile = xnT_h[:, b * S + it * 512: b * S + (it + 1) * 512]
                nc.tensor.matmul(
                    ps, lhsT=kz[:, h % 4, jc * 128:(jc + 1) * 128], rhs=qtile,
                    start=True, stop=False,
                )
                nc.tensor.matmul(
                    ps, lhsT=idh_sb[:, h, :],
                    rhs=bias_sb[:, jc, it * 512:(it + 1) * 512],
                    start=False, stop=True,
                )
                ps_q[u] = ps

            qkb(0)
            qkb(1)
            for u in range(16):
                jc, it = units[u]
                et = ep.tile([128, 512], BF16, tag="et")
                nc.scalar.activation(out=et, in_=ps_q[u], func=EXP, scale=SCALE)
                if u + 2 < 16:
                    qkb(u + 2)
                if 2 <= u <= 9:
                    flush_step()
                if u == hook_u and hook is not None:
                    hook()
                nc.tensor.matmul(
                    po[it], lhsT=v_aug[:, b * 8 + jc, h, 0:65], rhs=et,
                    start=(jc == 0), stop=(jc == S // 128 - 1),
                )
            # drain PV psum -> SBUF; defer PE transposes into next head
            ot = otp.tile([65, S], F32, tag="ot")
            for it in range(2):
                nc.vector.tensor_copy(
                    out=ot[:, it * 512:(it + 1) * 512], in_=po[it]
                )
            pending[0] = (b, h, ot, 0)
            if hook is not None:
                hook()

    def phase_d(bi, l, hook=None, prefetch=None):
        """MLP for batch bi (tokens bi*1024 .. +1024). hook() interleaves
        another phase's LN; first call comes after the spread flush."""
        can_hook = pending[0] is None
        last_tt = bi * 2 + 1
        for tt in range(bi * 2, bi * 2 + 2):
            hT = htp.tile([128, H2 // 128, 512], BF16, tag="hT")
            for hb in range(H2 // 128):
                flush_step()
                if can_hook and hook is not None:
                    hook()
                pm = ps_unit.tile([128, 512], F32, tag="unit", name="pm")
                for k in range(D // 128):
                    nc.tensor.matmul(
                        pm,
                        lhsT=w1_sb[:, l, k, hb * 128:(hb + 1) * 128],
                        rhs=xnT[k][:, tt * 512:(tt + 1) * 512],
                        start=(k == 0), stop=(k == D // 128 - 1),
                    )
                nc.scalar.activation(
                    out=hT[:, hb, :], in_=pm,
                    func=mybir.ActivationFunctionType.Gelu,
                    bias=b1_sb[:, l, hb:hb + 1],
                )
            can_hook = True  # pending fully flushed by first tt's hb loop
            if tt == last_tt and prefetch is not None:
                for h in range(3):
                    build_kz(prefetch, h)
                kz_ready[0] = prefetch
            for t2 in range(4):
                if hook is not None:
                    hook()
                pm2 = ps_small.tile([128, D], F32, tag="small", name="pm2")
                for hb in range(H2 // 128):
                    nc.tensor.matmul(
                        pm2,
                        lhsT=hT[:, hb, t2 * 128:(t2 + 1) * 128],
                        rhs=w2_sb[:, l, hb, :],
                        start=(hb == 0), stop=False,
                    )
                nc.tensor.matmul(
                    pm2, lhsT=ones128, rhs=b2_sb[:, l, :],
                    start=False, stop=True,
                )
                c = tt * 4 + t2
                nc.vector.tensor_add(
                    out=x_sb[:, c, :], in0=x_sb[:, c, :], in1=pm2
                )

    def lnf_cb(c, xn):
        nc.sync.dma_start(out=y[:, c, :], in_=xn)

    # ---- main schedule: batch-interleaved phases ----
    # Each LN phase is emitted chunk-wise via hooks inside the preceding
    # PE-heavy phase so its vector/scalar latency hides under matmuls.
    layer_norm_chunks(range(0, 8), (ln1w_sb[:, 0, :], ln1b_sb[:, 0, :]), a_cb)
    for l in range(L):
        phase_b(0, hook=phase_a_hook(1, l), prefetch=1)  # LN1(b1) in B(b0)
        phase_b(1, hook=phase_c_hook(0, l))   # LN2(b0) inside B(b1)
        phase_d(0, l, hook=phase_c_hook(1, l))  # LN2(b1) inside D(b0)
        if l + 1 < L:
            # LN1(b0,l+1) inside D(b1); prefetch next layer's kz tail-end
            phase_d(1, l, hook=phase_a_hook(0, l + 1), prefetch=0)
        else:
            # final LN of batch 0 (in place, per-chunk output DMA) in D(b1)
            phase_d(1, l, hook=ln_hook(0, (lnfw_sb, lnfb_sb), lnf_cb,
                                       inplace_into=lambda c: x_sb[:, c, :]))

    # ---- final LN for batch 1 (tail); in place, per-chunk DMA out ----
    layer_norm_chunks(range(8, NCH), (lnfw_sb, lnfb_sb), lnf_cb,
                      inplace_into=lambda c: x_sb[:, c, :])


def _install_ntff_hook():
    """Wire antenv.axon_hooks NTFF profiling via libaxon ctypes (dev only)."""
    if _cache.get("hook_done"):
        return
    _cache["hook_done"] = True
    try:
        import types
        import sys
        try:
            from antenv.axon_hooks import set_axon_ntff_profile_hook  # noqa
        except ImportError:
            import antenv
            mod = types.ModuleType("antenv.axon_hooks")
            holder = [None]
            mod.set_axon_ntff_profile_hook = lambda h: holder.__setitem__(0, h)
            mod.get_axon_ntff_profile_hook = lambda: holder[0]
            sys.modules["antenv.axon_hooks"] = mod
            antenv.axon_hooks = mod
            from trn_agent_boot.trn_boot import _ntff_profile_via_ctypes
            mod.set_axon_ntff_profile_hook(
                _ntff_profile_via_ctypes("/opt/axon/libaxon_pjrt.so"))
    except Exception as e:  # fail-soft: tracing degrades, run still works
        print("ntff hook install failed:", e)


def kernel(tokens, pos_ids, emb_table, input_weight, position_weight,
           ln1_w, ln1_b, ln2_w, ln2_b, w1, b1, w2, b2, lnf_w, lnf_b):
    tokens = np.asarray(tokens)
    pos_ids = np.asarray(pos_ids)
    emb_table = np.asarray(emb_table, dtype=np.float32)
    x0 = (np.float32(np.asarray(input_weight).reshape(-1)[0])
          * emb_table[tokens]
          + np.float32(np.asarray(position_weight).reshape(-1)[0])
          * _pos_table()[np.asarray(pos_ids)][None]).astype(np.float32)

    w1 = np.asarray(w1, np.float32)
    b1 = np.asarray(b1, np.float32)
    w2 = np.asarray(w2, np.float32)
    b2 = np.asarray(b2, np.float32)
    ln2_w = np.asarray(ln2_w, np.float32)
    ln2_b = np.asarray(ln2_b, np.float32)
    # fold LN2 affine into MLP weights
    w1eff = ln2_w[:, :, None] * w1                     # [L, D, H2]
    b1eff = b1 + np.einsum("ld,ldh->lh", ln2_b, w1)    # [L, H2]
    w1h = np.ascontiguousarray(
        w1eff.reshape(L, D // 128, 128, H2).transpose(2, 0, 1, 3))
    b1h = np.ascontiguousarray(
        b1eff.reshape(L, H2 // 128, 128).transpose(2, 0, 1))
    w2h = np.ascontiguousarray(
        w2.reshape(L, H2 // 128, 128, D).transpose(2, 0, 1, 3)
    ).astype(ml_dtypes.bfloat16)

    nc = _build_bass()
    base = {
        "biasT0": _alibi_bias0T(),
        "idh": _scaled_identities(),
        "w1h": w1h,
        "b1h": b1h,
        "w2h": w2h,
        "b2h": np.ascontiguousarray(np.broadcast_to(
            (b2 / 128.0)[None, :, :], (128, L, D))).astype(np.float32),
        "ln1w": np.ascontiguousarray(np.broadcast_to(
            np.asarray(ln1_w, np.float32)[None, :, :], (128, L, D))),
        "ln1b": np.ascontiguousarray(np.broadcast_to(
            np.asarray(ln1_b, np.float32)[None, :, :], (128, L, D))),
        "lnfw": np.ascontiguousarray(np.broadcast_to(
            np.asarray(lnf_w, np.float32)[None, :], (128, D))),
        "lnfb": np.ascontiguousarray(np.broadcast_to(
            np.asarray(lnf_b, np.float32)[None, :], (128, D))),
    }
    in_maps = []
    for core in range(NC):
        xc = x0[core * BL:(core + 1) * BL].reshape(T, D)
        xh = np.ascontiguousarray(
            xc.reshape(NCH, 128, D).transpose(1, 0, 2))
        m = dict(base)
        m["x0"] = xh
        in_maps.append(m)

    trace = os.environ.get("KERNEL_TRACE", "0") == "1"
    if trace:
        _install_ntff_hook()
    res = run_bass_kernel_spmd(
        nc, in_maps, core_ids=list(range(NC)), trace=trace,
        trace_cores=[0] if trace else None,
    )
    if trace and res.exec_time_ns is not None:
        print(f"HW exec time: {res.exec_time_ns} ns")
        if res.instructions_and_trace is not None:
            print("trace:", res.instructions_and_trace[1])

    out = np.empty((B, S, D), np.float32)
    for core in range(NC):
        yh = res.results[core]["y"]  # [128, NCH, D]
        yc = yh.transpose(1, 0, 2).reshape(BL, S, D)
        out[core * BL:(core + 1) * BL] = yc
    return out
